# revision 16
# baseline (speedup 1.0000x reference)
"""Trainium2 Bass kernel for nn_AttentionBlock64: batch of 8192 independent
64x64 attention tiles, data-parallel across 8 NeuronCores.

out[b] = (softmax(q[b] @ k[b]^T) @ v[b]) @ proj[b] + residual[b]

Math restructure (per tile, accumulation in fp32 PSUM):
  qT,kT = PE transpose (f32 in, 2cyc/row); PSUM->SBUF copy casts to fp16
  S   = mm(lhsT=qT, rhs=kT)          (fp16, PSUM f32)
  E   = exp(S) on ACT (bf16 out), accum_out giving row sums r for free
  Pr  = E * (1/r)  (ACT copy with per-partition scale AP = probs)
  PrT = PE transpose (bf16)
  U^T = mm(lhsT=v, rhs=PrT)          (bf16)
  out = mm(lhsT=U^T, rhs=proj) + residual   (DVE add straight from PSUM)

Engine balance (the previous version choked on GPSIMD casts at ~3.6ns/elem
and DVE broadcast-muls at ~1.7us each):
  GPSIMD: only the v cast. PE: transposes + matmuls (f32 transposes remove
  the q/k casts entirely; the PSUM evacuation casts to fp16 for free).
  ACT: kt copy, exp(+accum rowsum), probs scale-copies, et copy.
  DVE: qt/ut copies, proj cast, reciprocal, residual add.
  DMA: all HWDGE; sync ring: q,k,v; scalar ring: proj,res,store.

Tile bookkeeping for superblocks of 16 tiles (8 pairs, 4 quads):
  pair view: partition=(h i), tile t = 2*pb + h on partition half h.
  q/k transposed quad blocks: tile t at (partition half (t%4)//2, free
  half t%2, quad t//4).
  S/E slots chosen so that (a) E-transpose lands tile t on partition half
  t%2 (so M2's lhsT = v works straight from the pair view - one
  full-partition DMA) and (b) r's layout matches E's partitions for the
  probs scale: S_t lives at partition half t//8, free slot w(t), in bank
  A/B by (t%4)//2; E slot sl = t%8.
  M2/M3 run on diagonal quadrants (t%2, t%2): concurrent row-groups write
  disjoint partition halves of a single PSUM bank (same-bank same-partition
  concurrent PE writes wedge the device).
"""

import numpy as np

import concourse.bass as bass
import concourse.bacc as bacc
import concourse.mybir as mybir
from concourse import tile

F32 = mybir.dt.float32
F16 = mybir.dt.float16
BF16 = mybir.dt.bfloat16

T = 64          # tile dim
N_CORES = 8
B_FULL = 8192
NT_CORE = B_FULL // N_CORES   # tiles per core


def build(nt=NT_CORE, sbp=8, m1_dt=F16, m23_dt=BF16, in_bufs=7, mid_bufs=3,
          out_bufs=4):
    """Build the SPMD single-core program processing `nt` tiles."""
    sbt = 2 * sbp            # tiles per superblock
    nq = sbp // 2            # quads per superblock
    assert sbp == 8, "slot bookkeeping assumes 16-tile superblocks"
    assert nt % sbt == 0
    nsb = nt // sbt

    nc = bacc.Bacc("TRN2", target_bir_lowering=False, debug=False)

    q_d = nc.dram_tensor("q", [nt, T, T], F32, kind="ExternalInput").ap()
    k_d = nc.dram_tensor("k", [nt, T, T], F32, kind="ExternalInput").ap()
    v_d = nc.dram_tensor("v", [nt, T, T], F32, kind="ExternalInput").ap()
    p_d = nc.dram_tensor("proj", [nt, T, T], F32, kind="ExternalInput").ap()
    r_d = nc.dram_tensor("residual", [nt, T, T], F32, kind="ExternalInput").ap()
    o_d = nc.dram_tensor("out", [nt, T, T], F32, kind="ExternalOutput").ap()

    # pair view: partition = (h i); tile t = 2*pb + h at partition half h
    pairv = lambda ap: ap.rearrange("(ns pb h) i j -> ns (h i) pb j", pb=sbp, h=2)
    qv, kv, vv, pv, rv, ov = (pairv(x) for x in (q_d, k_d, v_d, p_d, r_d, o_d))

    # t -> S-psum position: single bank; partition half c1 = row-group rh
    # (diagonal quadrants -> concurrent M1 writes hit disjoint partitions),
    # free slot sl with sl%2 == t%2 (so the probs-transpose lands tile t on
    # partition half t%2, matching the v/proj/residual pair views).
    srh = lambda t: (t % 4) // 2
    sc1 = lambda t: (t % 4) // 2
    ssl = lambda t: (t // 4) * 2 + t % 2

    with tile.TileContext(nc) as tc:
        with (
            tc.tile_pool(name="consts", bufs=1) as consts,
            tc.tile_pool(name="qkin", bufs=in_bufs) as qkin,
            tc.tile_pool(name="vpin", bufs=in_bufs) as vpin,
            tc.tile_pool(name="resin", bufs=in_bufs) as resin,
            tc.tile_pool(name="in16", bufs=3) as in16,
            tc.tile_pool(name="mid", bufs=mid_bufs) as mid,
            tc.tile_pool(name="outp", bufs=out_bufs) as outp,
            tc.tile_pool(name="ps_tq", bufs=1, space="PSUM") as ps_tq,
            tc.tile_pool(name="ps_tk", bufs=1, space="PSUM") as ps_tk,
            tc.tile_pool(name="ps_s", bufs=2, space="PSUM") as ps_s,
            tc.tile_pool(name="ps_te", bufs=1, space="PSUM") as ps_te,
            tc.tile_pool(name="ps_u", bufs=1, space="PSUM") as ps_u,
            tc.tile_pool(name="ps_p", bufs=2, space="PSUM") as ps_p,
        ):
            # --- identity matrices for PE transposes ---
            id32 = consts.tile([128, 128], F32, tag="id32")
            idbf = consts.tile([128, 128], m23_dt, tag="idbf")
            ones32 = consts.tile([128, 128], F32, tag="ones32")
            onesbf = consts.tile([128, 128], m23_dt, tag="onesbf")
            nc.gpsimd.memset(ones32[:], 1.0)
            nc.gpsimd.memset(onesbf[:], 1.0)
            for src, dst in ((ones32, id32), (onesbf, idbf)):
                nc.gpsimd.affine_select(
                    out=dst[:], in_=src[:], pattern=[[-1, 128]],
                    compare_op=mybir.AluOpType.is_equal, fill=0.0,
                    base=0, channel_multiplier=1,
                )

            def front(s):
                """loads -> casts -> q/k transposes -> M1 -> exp -> probs.
                Returns the state back() needs one iteration later."""
                # ---- loads (HWDGE, f32): sync ring q,k,v; scalar p,res ----
                ql = qkin.tile([128, sbp, T], F32, tag="ql")
                kl = qkin.tile([128, sbp, T], F32, tag="kl")
                vl = vpin.tile([128, sbp, T], F32, tag="vl")
                pl = vpin.tile([128, sbp, T], F32, tag="pl")
                rl = resin.tile([128, sbp, T], F32, tag="rl")
                nc.sync.dma_start(out=ql[:], in_=qv[s])
                nc.sync.dma_start(out=kl[:], in_=kv[s])
                nc.scalar.dma_start(out=vl[:], in_=vv[s])
                nc.scalar.dma_start(out=pl[:], in_=pv[s])
                nc.sync.dma_start(out=rl[:], in_=rv[s])

                # ---- cast v, proj to bf16 on gpsimd (its only compute) ----
                vl16 = in16.tile([128, sbp, T], m23_dt, tag="vl16")
                pl16 = in16.tile([128, sbp, T], m23_dt, tag="pl16")
                nc.gpsimd.tensor_copy(vl16[:], vl[:])
                nc.vector.tensor_copy(pl16[:], pl[:])

                # ---- transpose q,k quads on PE in f32 (no input cast);
                #      the PSUM->SBUF copies cast to fp16 for M1 ----
                tq = ps_tq.tile([128, nq, 128], F32, tag="tq")
                tk = ps_tk.tile([128, nq, 128], F32, tag="tk")
                for g in range(nq):
                    nc.tensor.matmul(
                        tq[:, g, :], ql[:, 2 * g : 2 * g + 2, :], id32[:],
                        is_transpose=True,
                        start=(g == 0), stop=(g == nq - 1),
                    )
                for g in range(nq):
                    nc.tensor.matmul(
                        tk[:, g, :], kl[:, 2 * g : 2 * g + 2, :], id32[:],
                        is_transpose=True,
                        start=(g == 0), stop=(g == nq - 1),
                    )
                qt = mid.tile([128, nq, 128], m1_dt, tag="qt")
                kt = mid.tile([128, nq, 128], m1_dt, tag="kt")
                nc.vector.tensor_copy(qt[:], tq[:])
                nc.scalar.copy(kt[:], tk[:])

                # ---- M1: S = q @ k^T  (lhsT=qT, rhs=kT) ----
                # t-order alternates row-groups so LDWEIGHTS pulls ahead;
                # concurrent diagonal quadrants write disjoint partition
                # halves of the single S bank.
                s_ps = ps_s.tile([128, sbp, T], F32, tag="s_ps")
                for t in (0, 2, 1, 3, 4, 6, 5, 7, 8, 10, 9, 11, 12, 14, 13, 15):
                    g, rh, fh = t // 4, srh(t), t % 2
                    c1, sl = sc1(t), ssl(t)
                    nc.tensor.matmul(
                        s_ps[c1 * 64 : c1 * 64 + 64, sl, :],
                        qt[rh * 64 : rh * 64 + 64, g, fh * 64 : fh * 64 + 64],
                        kt[rh * 64 : rh * 64 + 64, g, fh * 64 : fh * 64 + 64],
                        start=True, stop=True,
                        tile_position=(rh * 64, c1 * 64),
                        skip_group_check=True,
                    )

                # ---- exp on ACT (one big inst), row sums + probs = E*(1/r)
                #      on DVE ----
                # e_sb[(c1,i), sl, j] = E_t[i,j] for the tile at (c1, sl)
                e_sb = mid.tile([128, sbp, T], m23_dt, tag="e_sb")
                nc.scalar.activation(
                    e_sb[:], s_ps[:],
                    mybir.ActivationFunctionType.Exp,
                )
                r_sb = mid.tile([128, sbp], F32, tag="r_sb")
                nc.vector.reduce_sum(out=r_sb[:], in_=e_sb[:],
                                     axis=mybir.AxisListType.X)
                rinv = mid.tile([128, sbp], F32, tag="rinv")
                nc.vector.reciprocal(rinv[:], r_sb[:])
                pr_sb = mid.tile([128, sbp, T], m23_dt, tag="pr_sb")
                nc.vector.tensor_mul(
                    pr_sb[:], e_sb[:], rinv[:].broadcast_to((128, sbp, T)))
                return s, pr_sb, vl16, pl16, rl

            def back(state):
                """probs transpose -> M2 -> M3 -> +residual -> store."""
                s, pr_sb, vl16, pl16, rl = state
                # ---- transpose probs quads on PE (bf16): tile t lands on
                #      partition half t%2 = its pair-view half ----
                te = ps_te.tile([128, nq, 128], m23_dt, tag="te")
                for G in range(nq):
                    nc.tensor.matmul(
                        te[:, G, :], pr_sb[:, 2 * G : 2 * G + 2, :], idbf[:],
                        is_transpose=True,
                        start=(G == 0), stop=(G == nq - 1),
                    )
                et = mid.tile([128, nq, 128], m23_dt, tag="et")
                nc.scalar.copy(et[:], te[:])

                # ---- M2: U^T = mm(lhsT=v, rhs=PrT) on diag quadrants;
                #      concurrent halves write disjoint partitions of u_ps
                u_ps = ps_u.tile([128, sbp, T], F32, tag="u_ps")
                for t in range(sbt):
                    h, pb = t % 2, t // 2
                    G, c1 = t // 4, (t % 4) // 2
                    nc.tensor.matmul(
                        u_ps[h * 64 : h * 64 + 64, pb, :],
                        vl16[h * 64 : h * 64 + 64, pb, :],
                        et[h * 64 : h * 64 + 64, G, c1 * 64 : c1 * 64 + 64],
                        start=True, stop=True,
                        tile_position=(h * 64, h * 64),
                        skip_group_check=True,
                    )
                ut = mid.tile([128, sbp, T], m23_dt, tag="ut")
                nc.scalar.copy(ut[:], u_ps[:])

                # ---- M3: P = mm(lhsT=U^T, rhs=proj), diag quadrants ----
                p_ps = ps_p.tile([128, sbp, T], F32, tag="p_ps")
                for t in range(sbt):
                    h, pb = t % 2, t // 2
                    nc.tensor.matmul(
                        p_ps[h * 64 : h * 64 + 64, pb, :],
                        ut[h * 64 : h * 64 + 64, pb, :],
                        pl16[h * 64 : h * 64 + 64, pb, :],
                        start=True, stop=True,
                        tile_position=(h * 64, h * 64),
                        skip_group_check=True,
                    )

                # ---- add residual straight from PSUM, store ----
                # store rides SWDGE (gpsimd) as a third DMA queue so a
                # store waiting on add() never head-of-line blocks the
                # HWDGE load rings.
                o_sb = outp.tile([128, sbp, T], F32, tag="o_sb")
                nc.vector.tensor_add(o_sb[:], p_ps[:], rl[:])
                nc.gpsimd.dma_start(out=ov[s], in_=o_sb[:])

            # software-pipelined emission: each iteration carries superblock
            # s's front half and superblock s-1's back half, so every
            # engine's program order interleaves the two (in particular DVE's
            # reduce/mul(s) are no longer queued behind add(s-1)'s M3 wait).
            pending = None
            for s in range(nsb):
                state = front(s)
                if pending is not None:
                    back(pending)
                pending = state
            back(pending)

    nc.compile()
    return nc


_BUILT = {}


def _get_built(nt=NT_CORE, sbp=8):
    key = (nt, sbp)
    if key not in _BUILT:
        _BUILT[key] = build(nt, sbp)
    return _BUILT[key]


def kernel(q, k, v, proj, residual):
    from concourse.bass_utils import run_bass_kernel_spmd

    q, k, v, proj, residual = (
        np.ascontiguousarray(np.asarray(x, dtype=np.float32))
        for x in (q, k, v, proj, residual)
    )
    nc = _get_built()
    nt = NT_CORE
    in_maps = []
    for c in range(N_CORES):
        sl = slice(c * nt, (c + 1) * nt)
        in_maps.append(
            {"q": q[sl], "k": k[sl], "v": v[sl], "proj": proj[sl],
             "residual": residual[sl]}
        )
    res = run_bass_kernel_spmd(nc, in_maps, list(range(N_CORES)))
    return np.concatenate([res.results[c]["out"] for c in range(N_CORES)], axis=0)


# revision 17
# speedup vs baseline: 1.0011x; 1.0011x over previous
"""Trainium2 Bass kernel for nn_AttentionBlock64: batch of 8192 independent
64x64 attention tiles, data-parallel across 8 NeuronCores.

out[b] = (softmax(q[b] @ k[b]^T) @ v[b]) @ proj[b] + residual[b]

Math restructure (per tile, accumulation in fp32 PSUM):
  qT,kT = PE transpose (f32 in, 2cyc/row); PSUM->SBUF copy casts to fp16
  S   = mm(lhsT=qT, rhs=kT)          (fp16, PSUM f32)
  E   = exp(S) on ACT (bf16 out), accum_out giving row sums r for free
  Pr  = E * (1/r)  (ACT copy with per-partition scale AP = probs)
  PrT = PE transpose (bf16)
  U^T = mm(lhsT=v, rhs=PrT)          (bf16)
  out = mm(lhsT=U^T, rhs=proj) + residual   (DVE add straight from PSUM)

Engine balance (the previous version choked on GPSIMD casts at ~3.6ns/elem
and DVE broadcast-muls at ~1.7us each):
  GPSIMD: only the v cast. PE: transposes + matmuls (f32 transposes remove
  the q/k casts entirely; the PSUM evacuation casts to fp16 for free).
  ACT: kt copy, exp(+accum rowsum), probs scale-copies, et copy.
  DVE: qt/ut copies, proj cast, reciprocal, residual add.
  DMA: all HWDGE; sync ring: q,k,v; scalar ring: proj,res,store.

Tile bookkeeping for superblocks of 16 tiles (8 pairs, 4 quads):
  pair view: partition=(h i), tile t = 2*pb + h on partition half h.
  q/k transposed quad blocks: tile t at (partition half (t%4)//2, free
  half t%2, quad t//4).
  S/E slots chosen so that (a) E-transpose lands tile t on partition half
  t%2 (so M2's lhsT = v works straight from the pair view - one
  full-partition DMA) and (b) r's layout matches E's partitions for the
  probs scale: S_t lives at partition half t//8, free slot w(t), in bank
  A/B by (t%4)//2; E slot sl = t%8.
  M2/M3 run on diagonal quadrants (t%2, t%2): concurrent row-groups write
  disjoint partition halves of a single PSUM bank (same-bank same-partition
  concurrent PE writes wedge the device).
"""

import numpy as np

import concourse.bass as bass
import concourse.bacc as bacc
import concourse.mybir as mybir
from concourse import tile

F32 = mybir.dt.float32
F16 = mybir.dt.float16
BF16 = mybir.dt.bfloat16

T = 64          # tile dim
N_CORES = 8
B_FULL = 8192
NT_CORE = B_FULL // N_CORES   # tiles per core


def build(nt=NT_CORE, sbp=8, m1_dt=F16, m23_dt=BF16, in_bufs=7, mid_bufs=3,
          out_bufs=4):
    """Build the SPMD single-core program processing `nt` tiles."""
    sbt = 2 * sbp            # tiles per superblock
    nq = sbp // 2            # quads per superblock
    assert sbp == 8, "slot bookkeeping assumes 16-tile superblocks"
    assert nt % sbt == 0
    nsb = nt // sbt

    nc = bacc.Bacc("TRN2", target_bir_lowering=False, debug=False)

    q_d = nc.dram_tensor("q", [nt, T, T], F32, kind="ExternalInput").ap()
    k_d = nc.dram_tensor("k", [nt, T, T], F32, kind="ExternalInput").ap()
    v_d = nc.dram_tensor("v", [nt, T, T], F32, kind="ExternalInput").ap()
    p_d = nc.dram_tensor("proj", [nt, T, T], F32, kind="ExternalInput").ap()
    r_d = nc.dram_tensor("residual", [nt, T, T], F32, kind="ExternalInput").ap()
    o_d = nc.dram_tensor("out", [nt, T, T], F32, kind="ExternalOutput").ap()

    # pair view: partition = (h i); tile t = 2*pb + h at partition half h
    pairv = lambda ap: ap.rearrange("(ns pb h) i j -> ns (h i) pb j", pb=sbp, h=2)
    qv, kv, vv, pv, rv, ov = (pairv(x) for x in (q_d, k_d, v_d, p_d, r_d, o_d))

    # t -> S-psum position: single bank; partition half c1 = row-group rh
    # (diagonal quadrants -> concurrent M1 writes hit disjoint partitions),
    # free slot sl with sl%2 == t%2 (so the probs-transpose lands tile t on
    # partition half t%2, matching the v/proj/residual pair views).
    srh = lambda t: (t % 4) // 2
    sc1 = lambda t: (t % 4) // 2
    ssl = lambda t: (t // 4) * 2 + t % 2

    with tile.TileContext(nc) as tc:
        with (
            tc.tile_pool(name="consts", bufs=1) as consts,
            tc.tile_pool(name="qkin", bufs=in_bufs) as qkin,
            tc.tile_pool(name="vpin", bufs=in_bufs) as vpin,
            tc.tile_pool(name="resin", bufs=in_bufs) as resin,
            tc.tile_pool(name="in16", bufs=3) as in16,
            tc.tile_pool(name="mid", bufs=mid_bufs) as mid,
            tc.tile_pool(name="outp", bufs=out_bufs) as outp,
            tc.tile_pool(name="ps_tq", bufs=1, space="PSUM") as ps_tq,
            tc.tile_pool(name="ps_tk", bufs=1, space="PSUM") as ps_tk,
            tc.tile_pool(name="ps_s", bufs=2, space="PSUM") as ps_s,
            tc.tile_pool(name="ps_te", bufs=1, space="PSUM") as ps_te,
            tc.tile_pool(name="ps_u", bufs=1, space="PSUM") as ps_u,
            tc.tile_pool(name="ps_p", bufs=2, space="PSUM") as ps_p,
        ):
            # --- identity matrices for PE transposes ---
            id32 = consts.tile([128, 128], F32, tag="id32")
            idbf = consts.tile([128, 128], m23_dt, tag="idbf")
            ones32 = consts.tile([128, 128], F32, tag="ones32")
            onesbf = consts.tile([128, 128], m23_dt, tag="onesbf")
            nc.gpsimd.memset(ones32[:], 1.0)
            nc.gpsimd.memset(onesbf[:], 1.0)
            for src, dst in ((ones32, id32), (onesbf, idbf)):
                nc.gpsimd.affine_select(
                    out=dst[:], in_=src[:], pattern=[[-1, 128]],
                    compare_op=mybir.AluOpType.is_equal, fill=0.0,
                    base=0, channel_multiplier=1,
                )

            def front(s):
                """loads -> casts -> q/k transposes -> M1 -> exp -> probs.
                Returns the state back() needs one iteration later."""
                # ---- loads (HWDGE, f32): sync ring q,k,v; scalar p,res ----
                ql = qkin.tile([128, sbp, T], F32, tag="ql")
                kl = qkin.tile([128, sbp, T], F32, tag="kl")
                vl = vpin.tile([128, sbp, T], F32, tag="vl")
                pl = vpin.tile([128, sbp, T], F32, tag="pl")
                rl = resin.tile([128, sbp, T], F32, tag="rl")
                nc.sync.dma_start(out=ql[:], in_=qv[s])
                nc.sync.dma_start(out=kl[:], in_=kv[s])
                nc.scalar.dma_start(out=vl[:], in_=vv[s])
                nc.scalar.dma_start(out=pl[:], in_=pv[s])
                nc.sync.dma_start(out=rl[:], in_=rv[s])

                # ---- cast v, proj to bf16 on gpsimd (its only compute) ----
                vl16 = in16.tile([128, sbp, T], m23_dt, tag="vl16")
                pl16 = in16.tile([128, sbp, T], m23_dt, tag="pl16")
                nc.gpsimd.tensor_copy(vl16[:], vl[:])
                nc.vector.tensor_copy(pl16[:], pl[:])

                # ---- transpose q,k quads on PE in f32 (no input cast);
                #      the PSUM->SBUF copies cast to fp16 for M1 ----
                tq = ps_tq.tile([128, nq, 128], F32, tag="tq")
                tk = ps_tk.tile([128, nq, 128], F32, tag="tk")
                for g in range(nq):
                    nc.tensor.matmul(
                        tq[:, g, :], ql[:, 2 * g : 2 * g + 2, :], id32[:],
                        is_transpose=True,
                        start=(g == 0), stop=(g == nq - 1),
                    )
                for g in range(nq):
                    nc.tensor.matmul(
                        tk[:, g, :], kl[:, 2 * g : 2 * g + 2, :], id32[:],
                        is_transpose=True,
                        start=(g == 0), stop=(g == nq - 1),
                    )
                qt = mid.tile([128, nq, 128], m1_dt, tag="qt")
                kt = mid.tile([128, nq, 128], m1_dt, tag="kt")
                nc.scalar.copy(qt[:], tq[:])
                nc.scalar.copy(kt[:], tk[:])

                # ---- M1: S = q @ k^T  (lhsT=qT, rhs=kT) ----
                # t-order alternates row-groups so LDWEIGHTS pulls ahead;
                # concurrent diagonal quadrants write disjoint partition
                # halves of the single S bank.
                s_ps = ps_s.tile([128, sbp, T], F32, tag="s_ps")
                for t in (0, 2, 1, 3, 4, 6, 5, 7, 8, 10, 9, 11, 12, 14, 13, 15):
                    g, rh, fh = t // 4, srh(t), t % 2
                    c1, sl = sc1(t), ssl(t)
                    nc.tensor.matmul(
                        s_ps[c1 * 64 : c1 * 64 + 64, sl, :],
                        qt[rh * 64 : rh * 64 + 64, g, fh * 64 : fh * 64 + 64],
                        kt[rh * 64 : rh * 64 + 64, g, fh * 64 : fh * 64 + 64],
                        start=True, stop=True,
                        tile_position=(rh * 64, c1 * 64),
                        skip_group_check=True,
                    )

                # ---- exp on ACT (one big inst), row sums + probs = E*(1/r)
                #      on DVE ----
                # e_sb[(c1,i), sl, j] = E_t[i,j] for the tile at (c1, sl)
                e_sb = mid.tile([128, sbp, T], m23_dt, tag="e_sb")
                nc.scalar.activation(
                    e_sb[:], s_ps[:],
                    mybir.ActivationFunctionType.Exp,
                )
                r_sb = mid.tile([128, sbp], F32, tag="r_sb")
                nc.vector.reduce_sum(out=r_sb[:], in_=e_sb[:],
                                     axis=mybir.AxisListType.X)
                rinv = mid.tile([128, sbp], F32, tag="rinv")
                nc.vector.reciprocal(rinv[:], r_sb[:])
                rinv16 = mid.tile([128, sbp], m23_dt, tag="rinv16")
                nc.vector.tensor_copy(rinv16[:], rinv[:])
                pr_sb = mid.tile([128, sbp, T], m23_dt, tag="pr_sb")
                nc.vector.tensor_mul(
                    pr_sb[:], e_sb[:], rinv16[:].broadcast_to((128, sbp, T)))
                return s, pr_sb, vl16, pl16, rl

            def back(state):
                """probs transpose -> M2 -> M3 -> +residual -> store."""
                s, pr_sb, vl16, pl16, rl = state
                # ---- transpose probs quads on PE (bf16): tile t lands on
                #      partition half t%2 = its pair-view half ----
                te = ps_te.tile([128, nq, 128], m23_dt, tag="te")
                for G in range(nq):
                    nc.tensor.matmul(
                        te[:, G, :], pr_sb[:, 2 * G : 2 * G + 2, :], idbf[:],
                        is_transpose=True,
                        start=(G == 0), stop=(G == nq - 1),
                    )
                et = mid.tile([128, nq, 128], m23_dt, tag="et")
                nc.scalar.copy(et[:], te[:])

                # ---- M2: U^T = mm(lhsT=v, rhs=PrT) on diag quadrants;
                #      concurrent halves write disjoint partitions of u_ps
                u_ps = ps_u.tile([128, sbp, T], F32, tag="u_ps")
                for t in range(sbt):
                    h, pb = t % 2, t // 2
                    G, c1 = t // 4, (t % 4) // 2
                    nc.tensor.matmul(
                        u_ps[h * 64 : h * 64 + 64, pb, :],
                        vl16[h * 64 : h * 64 + 64, pb, :],
                        et[h * 64 : h * 64 + 64, G, c1 * 64 : c1 * 64 + 64],
                        start=True, stop=True,
                        tile_position=(h * 64, h * 64),
                        skip_group_check=True,
                    )
                ut = mid.tile([128, sbp, T], m23_dt, tag="ut")
                nc.scalar.copy(ut[:], u_ps[:])

                # ---- M3: P = mm(lhsT=U^T, rhs=proj), diag quadrants ----
                p_ps = ps_p.tile([128, sbp, T], F32, tag="p_ps")
                for t in range(sbt):
                    h, pb = t % 2, t // 2
                    nc.tensor.matmul(
                        p_ps[h * 64 : h * 64 + 64, pb, :],
                        ut[h * 64 : h * 64 + 64, pb, :],
                        pl16[h * 64 : h * 64 + 64, pb, :],
                        start=True, stop=True,
                        tile_position=(h * 64, h * 64),
                        skip_group_check=True,
                    )

                # ---- add residual straight from PSUM, store ----
                # store rides SWDGE (gpsimd) as a third DMA queue so a
                # store waiting on add() never head-of-line blocks the
                # HWDGE load rings.
                o_sb = outp.tile([128, sbp, T], F32, tag="o_sb")
                nc.vector.tensor_add(o_sb[:], p_ps[:], rl[:])
                nc.gpsimd.dma_start(out=ov[s], in_=o_sb[:])

            # software-pipelined emission: each iteration carries superblock
            # s's front half and superblock s-1's back half, so every
            # engine's program order interleaves the two (in particular DVE's
            # reduce/mul(s) are no longer queued behind add(s-1)'s M3 wait).
            pending = None
            for s in range(nsb):
                state = front(s)
                if pending is not None:
                    back(pending)
                pending = state
            back(pending)

    nc.compile()
    return nc


_BUILT = {}


def _get_built(nt=NT_CORE, sbp=8):
    key = (nt, sbp)
    if key not in _BUILT:
        _BUILT[key] = build(nt, sbp)
    return _BUILT[key]


def kernel(q, k, v, proj, residual):
    from concourse.bass_utils import run_bass_kernel_spmd

    q, k, v, proj, residual = (
        np.ascontiguousarray(np.asarray(x, dtype=np.float32))
        for x in (q, k, v, proj, residual)
    )
    nc = _get_built()
    nt = NT_CORE
    in_maps = []
    for c in range(N_CORES):
        sl = slice(c * nt, (c + 1) * nt)
        in_maps.append(
            {"q": q[sl], "k": k[sl], "v": v[sl], "proj": proj[sl],
             "residual": residual[sl]}
        )
    res = run_bass_kernel_spmd(nc, in_maps, list(range(N_CORES)))
    return np.concatenate([res.results[c]["out"] for c in range(N_CORES)], axis=0)
